# revision 17
# baseline (speedup 1.0000x reference)
"""ChannelDense Trainium2 kernel (nn_ChannelDense_69045894250869).

y[b, n, :] = tanh(x[b, n, :] @ weight[channels[n]].T + bias[channels[n]]) + x[b, n, :]

Shapes: x [128, 2048, 256] f32, channels [2048] int64 (values < 64),
        weight [64, 256, 256] f32, bias [64, 256] f32.
Returns (y [128, 2048, 256] f32, channels passthrough), matching the reference.

Strategy (8 NeuronCores, SPMD single program):
  - Points (n axis) are sorted by channel, each channel's point list is padded
    to a multiple of 8 (repeating real points), and the padded list is dealt
    round-robin across the 8 cores.  Slot t on every core then has the same
    channel, so one static program works for all cores; all per-core
    variability lives in the host-sliced inputs (sharding by point list, with
    channels co-sharded, per the problem's sharding hint).
  - The whole device computation runs in the TRANSPOSED domain: the host
    ships each core's point shard as x^T [ichunk, i%128, slot, b] so the
    contraction axis i lies on SBUF partitions.  Per pair of same-channel
    slots: four bf16 matmuls (stationary = weight block [i,o], moving = x^T
    [i, (slot,b)] with N=256) accumulate z^T in fp32 PSUM; ACT applies
    tanh with the per-partition channel bias fused (z^T has o on partitions);
    DVE adds the x^T residual for a whole 8-slot chunk in one op; results
    store as y^T, which the host un-transposes while unsharding.
  - This keeps the PE free of transpose-mode instructions (which hold the
    HAM clock gate at 1.2 GHz) and needs no on-chip transposes, PSUM->SBUF
    copies, or bias matmuls at all.
  - XT_DTYPE selects what the x^T shard ships as: "bf16" (fastest: half the
    x traffic; the residual also uses bf16-rounded x, rel err ~3e-3) or
    "f32" (exact residual, rel err ~1e-3, more DMA).
"""

import os
import sys

sys.path.insert(0, "/opt/trn_rl_repo")
os.environ.setdefault("NEURON_RT_RESET_CORES", "1")

import numpy as np

import concourse.bass as bass  # noqa: F401
import concourse.mybir as mybir
import concourse.tile as tile
from concourse import bacc
from concourse.bass_utils import run_bass_kernel_spmd

F32 = mybir.dt.float32
BF16 = mybir.dt.bfloat16

XT_DTYPE = os.environ.get("CHD_XT", "bf16")  # "bf16" | "f32"
Y_DTYPE = os.environ.get("CHD_Y", "bf16")  # "bf16" | "f32"

N_CORES = 8
B = 128
N_POINTS = 2048
D_IN = 256
D_OUT = 256
N_CH = 64

_CACHE = {}


def _plan(channels: np.ndarray):
    """Sort points by channel, pad each channel to a multiple of 8, deal
    round-robin over cores.  Returns (slots_per_core T, deal [T,8], g [T])."""
    ch = np.asarray(channels).astype(np.int64)
    per_ch = {}
    for c in range(N_CH):
        pts = np.nonzero(ch == c)[0]
        if len(pts) == 0:
            continue
        pad = (-len(pts)) % N_CORES
        per_ch[c] = np.concatenate([pts, np.repeat(pts[-1], pad)])
    # order channels so even-slot-length runs come first: fewer slot pairs
    # that straddle a channel boundary at odd parity
    order = sorted(per_ch, key=lambda c: (len(per_ch[c]) // N_CORES) % 2)
    padded = []
    slot_ch = []
    for c in order:
        full = per_ch[c]
        padded.append(full)
        slot_ch.extend([c] * (len(full) // N_CORES))
    padded = np.concatenate(padded)
    assert len(padded) % N_CORES == 0
    T = len(padded) // N_CORES
    deal = padded.reshape(T, N_CORES)
    g = np.asarray(slot_ch, dtype=np.int64)
    assert len(g) == T
    return T, deal, g


def _build(T: int, g: np.ndarray):
    assert T % 8 == 0
    xt_bf16 = XT_DTYPE == "bf16"
    XDT = BF16 if xt_bf16 else F32
    YDT = BF16 if Y_DTYPE == "bf16" else F32
    nc = bacc.Bacc("TRN2", target_bir_lowering=False, debug=False, num_devices=N_CORES)

    # x^T shard: [ichunk, i%128, slot, b]
    xt_d = nc.dram_tensor("xt", [2, 128, T, B], XDT, kind="ExternalInput").ap()
    wt_d = nc.dram_tensor("wt", [N_CH, D_IN, D_OUT], BF16, kind="ExternalInput").ap()
    # bias: [o%128, 2*c + ochunk]
    bias_d = nc.dram_tensor("bias", [128, 2 * N_CH], F32, kind="ExternalInput").ap()
    # y^T: [ochunk, o%128, slot, b]
    y_d = nc.dram_tensor("y", [2, 128, T, B], YDT, kind="ExternalOutput").ap()

    TanhF = mybir.ActivationFunctionType.Tanh

    with tile.TileContext(nc) as tc:
        with (
            tc.tile_pool(name="const", bufs=1) as cpool,
            tc.tile_pool(name="xin", bufs=12) as xpool,
            tc.tile_pool(name="xbq", bufs=4) as xbpool,
            tc.tile_pool(name="out", bufs=10) as opool,
            tc.tile_pool(name="yps", bufs=8, space="PSUM") as yps_pool,
        ):
            bias_sb = cpool.tile([128, 2 * N_CH], F32, tag="bias")
            nc.sync.dma_start(bias_sb[:], bias_d)

            # all 64 transposed weights resident: [i%128, (c, ichunk, o)]
            wall_sb = cpool.tile([128, N_CH * 2 * D_OUT], BF16, tag="wall")
            nc.sync.dma_start(
                wall_sb[:].rearrange("p (c i o) -> p c i o", c=N_CH, i=2),
                wt_d.rearrange("c (i p) o -> p c i o", p=128),
            )

            def w_block(c, ic, oc):
                base = c * 512 + ic * 256 + oc * 128
                return wall_sb[:, base : base + 128]

            for j0 in range(T // 8):
                # x^T chunk: [128, (ic, s8, b)]
                xt_sb = xpool.tile([128, 2 * 8 * B], XDT, tag="x")
                ld_eng = nc.sync if j0 % 2 == 0 else nc.gpsimd
                st_eng = nc.gpsimd if j0 % 2 == 0 else nc.sync
                ld_eng.dma_start(
                    xt_sb[:].rearrange("p (c s b) -> p c s b", c=2, s=8),
                    xt_d[:, :, 8 * j0 : 8 * j0 + 8, :].rearrange("c p s b -> p c s b"),
                )
                if xt_bf16:
                    xmm = xt_sb
                else:
                    xmm = xbpool.tile([128, 2 * 8 * B], BF16, tag="xb")
                    nc.vector.tensor_copy(xmm[:], xt_sb[:])

                # o_sb: [128, (oc, s8, b)]
                o_sb = opool.tile([128, 2 * 8 * B], YDT, tag="out")

                for p in range(4):  # pairs of slots
                    sa, sb_ = 8 * j0 + 2 * p, 8 * j0 + 2 * p + 1
                    ca, cb = int(g[sa]), int(g[sb_])

                    # psum pair: [128, (oc, s2, b)]
                    y_ps = yps_pool.tile([128, 2 * 2 * B], F32, tag="yps")

                    if ca == cb:
                        # fused: 4 matmuls, N=256 (both slots)
                        mm = [(oc, ic) for oc in range(2) for ic in range(2)]
                        for idx, (oc, ic) in enumerate(mm):
                            nc.tensor.matmul(
                                y_ps[:, oc * 256 : oc * 256 + 256],
                                w_block(ca, ic, oc),
                                xmm[:, ic * 1024 + 2 * p * B : ic * 1024 + 2 * p * B + 256],
                                start=(idx == 0),
                                stop=(idx == len(mm) - 1),
                            )
                    else:
                        # boundary pair: per-slot matmuls, N=128
                        mm = [
                            (s_off, w_c, oc, ic)
                            for s_off, w_c in ((0, ca), (1, cb))
                            for oc in range(2)
                            for ic in range(2)
                        ]
                        for idx, (s_off, w_c, oc, ic) in enumerate(mm):
                            nc.tensor.matmul(
                                y_ps[:, oc * 256 + s_off * 128 : oc * 256 + s_off * 128 + 128],
                                w_block(w_c, ic, oc),
                                xmm[:, ic * 1024 + (2 * p + s_off) * B : ic * 1024 + (2 * p + s_off) * B + 128],
                                start=(idx == 0),
                                stop=(idx == len(mm) - 1),
                            )

                    # tanh + fused per-partition bias, psum -> o_sb
                    for oc in range(2):
                        if ca == cb:
                            nc.scalar.activation(
                                o_sb[:, oc * 1024 + 2 * p * B : oc * 1024 + 2 * p * B + 256],
                                y_ps[:, oc * 256 : oc * 256 + 256],
                                TanhF,
                                bias=bias_sb[:, 2 * ca + oc : 2 * ca + oc + 1],
                            )
                        else:
                            for s_off, c in ((0, ca), (1, cb)):
                                nc.scalar.activation(
                                    o_sb[:, oc * 1024 + (2 * p + s_off) * B : oc * 1024 + (2 * p + s_off) * B + 128],
                                    y_ps[:, oc * 256 + s_off * 128 : oc * 256 + s_off * 128 + 128],
                                    TanhF,
                                    bias=bias_sb[:, 2 * c + oc : 2 * c + oc + 1],
                                )

                    if p in (1, 3):
                        h0 = (p // 2) * 4 * B
                        o_h = o_sb[:].rearrange("q (c sb) -> q c sb", c=2)[
                            :, :, h0 : h0 + 4 * B
                        ]
                        x_h = xt_sb[:].rearrange("q (c sb) -> q c sb", c=2)[
                            :, :, h0 : h0 + 4 * B
                        ]
                        nc.vector.tensor_add(o_h, o_h, x_h)


                st_eng.dma_start(
                    y_d[:, :, 8 * j0 : 8 * j0 + 8, :].rearrange("c p s b -> p c s b"),
                    o_sb[:].rearrange("p (c s b) -> p c s b", c=2, s=8),
                )

    nc.compile()
    return nc


def _get_program(channels: np.ndarray):
    key = (XT_DTYPE, Y_DTYPE, np.asarray(channels).astype(np.int64).tobytes())
    if key not in _CACHE:
        T, deal, g = _plan(channels)
        if T % 8 != 0:
            pad = 8 - T % 8
            deal = np.concatenate([deal, np.repeat(deal[-1:], pad, axis=0)], axis=0)
            g = np.concatenate([g, np.repeat(g[-1:], pad)])
            T += pad
        nc = _build(T, g)
        _CACHE[key] = (T, deal, g, nc)
    return _CACHE[key]


def run(x, channels, weight, bias, trace=False):
    import ml_dtypes

    x = np.ascontiguousarray(np.asarray(x, dtype=np.float32))
    weight = np.asarray(weight, dtype=np.float32)
    bias = np.asarray(bias, dtype=np.float32)

    T, deal, g, nc = _get_program(channels)

    xdt = ml_dtypes.bfloat16 if XT_DTYPE == "bf16" else np.float32
    wt = np.ascontiguousarray(weight.transpose(0, 2, 1)).astype(ml_dtypes.bfloat16)
    # bias -> [o%128, 2*c + oc]
    bias_t = np.ascontiguousarray(
        bias.reshape(N_CH, 2, 128).transpose(2, 0, 1).reshape(128, 2 * N_CH)
    )

    in_maps = []
    for k in range(N_CORES):
        pts = deal[:, k]
        xg = x[:, pts, :]  # [B, T, 256]
        xtk = np.ascontiguousarray(xg.transpose(2, 1, 0)).reshape(2, 128, T, B)
        in_maps.append(
            {
                "xt": xtk.astype(xdt),
                "wt": wt,
                "bias": bias_t,
            }
        )

    res = run_bass_kernel_spmd(nc, in_maps, list(range(N_CORES)), trace=trace)

    y = np.empty((B, N_POINTS, D_OUT), dtype=np.float32)
    for k in range(N_CORES):
        # y^T [2, 128, T, B] -> [B, T, 256]
        ytk = np.asarray(res.results[k]["y"], dtype=np.float32).reshape(D_OUT, T, B)
        y[:, deal[:, k], :] = ytk.transpose(2, 1, 0)
    return y, res


def kernel(x, channels, weight, bias):
    y, _ = run(x, channels, weight, bias, trace=False)
    ch = np.asarray(channels)
    return (y, ch)


# revision 18
# speedup vs baseline: 1.0437x; 1.0437x over previous
"""ChannelDense Trainium2 kernel (nn_ChannelDense_69045894250869).

y[b, n, :] = tanh(x[b, n, :] @ weight[channels[n]].T + bias[channels[n]]) + x[b, n, :]

Shapes: x [128, 2048, 256] f32, channels [2048] int64 (values < 64),
        weight [64, 256, 256] f32, bias [64, 256] f32.
Returns (y [128, 2048, 256] f32, channels passthrough), matching the reference.

Strategy (8 NeuronCores, SPMD single program):
  - Points (n axis) are sorted by channel, each channel's point list is padded
    to a multiple of 8 (repeating real points), and the padded list is dealt
    round-robin across the 8 cores.  Slot t on every core then has the same
    channel, so one static program works for all cores; all per-core
    variability lives in the host-sliced inputs (sharding by point list, with
    channels co-sharded, per the problem's sharding hint).
  - The whole device computation runs in the TRANSPOSED domain: the host
    ships each core's point shard as x^T [ichunk, i%128, slot, b] so the
    contraction axis i lies on SBUF partitions.  Per pair of same-channel
    slots: four bf16 matmuls (stationary = weight block [i,o], moving = x^T
    [i, (slot,b)] with N=256) accumulate z^T in fp32 PSUM; ACT applies
    tanh with the per-partition channel bias fused (z^T has o on partitions);
    DVE adds the x^T residual for a whole 8-slot chunk in one op; results
    store as y^T, which the host un-transposes while unsharding.
  - This keeps the PE free of transpose-mode instructions (which hold the
    HAM clock gate at 1.2 GHz) and needs no on-chip transposes, PSUM->SBUF
    copies, or bias matmuls at all.
  - XT_DTYPE selects what the x^T shard ships as: "bf16" (fastest: half the
    x traffic; the residual also uses bf16-rounded x, rel err ~3e-3) or
    "f32" (exact residual, rel err ~1e-3, more DMA).
"""

import os
import sys

sys.path.insert(0, "/opt/trn_rl_repo")
os.environ.setdefault("NEURON_RT_RESET_CORES", "1")

import numpy as np

import concourse.bass as bass  # noqa: F401
import concourse.mybir as mybir
import concourse.tile as tile
from concourse import bacc
from concourse.bass_utils import run_bass_kernel_spmd

F32 = mybir.dt.float32
BF16 = mybir.dt.bfloat16

XT_DTYPE = os.environ.get("CHD_XT", "bf16")  # "bf16" | "f32"
Y_DTYPE = os.environ.get("CHD_Y", "bf16")  # "bf16" | "f32"

N_CORES = 8
B = 128
N_POINTS = 2048
D_IN = 256
D_OUT = 256
N_CH = 64

_CACHE = {}


def _plan(channels: np.ndarray):
    """Sort points by channel, pad each channel to a multiple of 8, deal
    round-robin over cores.  Returns (slots_per_core T, deal [T,8], g [T])."""
    ch = np.asarray(channels).astype(np.int64)
    per_ch = {}
    for c in range(N_CH):
        pts = np.nonzero(ch == c)[0]
        if len(pts) == 0:
            continue
        pad = (-len(pts)) % N_CORES
        per_ch[c] = np.concatenate([pts, np.repeat(pts[-1], pad)])
    # order channels so even-slot-length runs come first: fewer slot pairs
    # that straddle a channel boundary at odd parity
    order = sorted(per_ch, key=lambda c: (len(per_ch[c]) // N_CORES) % 2)
    padded = []
    slot_ch = []
    for c in order:
        full = per_ch[c]
        padded.append(full)
        slot_ch.extend([c] * (len(full) // N_CORES))
    padded = np.concatenate(padded)
    assert len(padded) % N_CORES == 0
    T = len(padded) // N_CORES
    deal = padded.reshape(T, N_CORES)
    g = np.asarray(slot_ch, dtype=np.int64)
    assert len(g) == T
    return T, deal, g


def _build(T: int, g: np.ndarray):
    assert T % 8 == 0
    xt_bf16 = XT_DTYPE == "bf16"
    XDT = BF16 if xt_bf16 else F32
    YDT = BF16 if Y_DTYPE == "bf16" else F32
    nc = bacc.Bacc("TRN2", target_bir_lowering=False, debug=False, num_devices=N_CORES)

    # x^T shard: [ichunk, i%128, slot, b]
    xt_d = nc.dram_tensor("xt", [2, 128, T, B], XDT, kind="ExternalInput").ap()
    wt_d = nc.dram_tensor("wt", [N_CH, D_IN, D_OUT], BF16, kind="ExternalInput").ap()
    # bias: [o%128, 2*c + ochunk]
    bias_d = nc.dram_tensor("bias", [128, 2 * N_CH], F32, kind="ExternalInput").ap()
    # y^T: [ochunk, o%128, slot, b]
    y_d = nc.dram_tensor("y", [2, 128, T, B], YDT, kind="ExternalOutput").ap()

    TanhF = mybir.ActivationFunctionType.Tanh

    with tile.TileContext(nc) as tc:
        with (
            tc.tile_pool(name="const", bufs=1) as cpool,
            tc.tile_pool(name="xin", bufs=12) as xpool,
            tc.tile_pool(name="xbq", bufs=4) as xbpool,
            tc.tile_pool(name="out", bufs=10) as opool,
            tc.tile_pool(name="yps", bufs=8, space="PSUM") as yps_pool,
        ):
            bias_sb = cpool.tile([128, 2 * N_CH], F32, tag="bias")
            nc.sync.dma_start(bias_sb[:], bias_d)

            # all 64 transposed weights resident: [i%128, (c, ichunk, o)]
            wall_sb = cpool.tile([128, N_CH * 2 * D_OUT], BF16, tag="wall")
            nc.sync.dma_start(
                wall_sb[:].rearrange("p (c i o) -> p c i o", c=N_CH, i=2),
                wt_d.rearrange("c (i p) o -> p c i o", p=128),
            )

            def w_block(c, ic, oc):
                base = c * 512 + ic * 256 + oc * 128
                return wall_sb[:, base : base + 128]

            for j0 in range(T // 8):
                # x^T chunk: [128, (ic, s8, b)]
                xt_sb = xpool.tile([128, 2 * 8 * B], XDT, tag="x")
                nc.sync.dma_start(
                    xt_sb[:].rearrange("p (c s b) -> p c s b", c=2, s=8),
                    xt_d[:, :, 8 * j0 : 8 * j0 + 8, :].rearrange("c p s b -> p c s b"),
                )
                if xt_bf16:
                    xmm = xt_sb
                else:
                    xmm = xbpool.tile([128, 2 * 8 * B], BF16, tag="xb")
                    nc.vector.tensor_copy(xmm[:], xt_sb[:])

                # o_sb: [128, (oc, s8, b)]
                o_sb = opool.tile([128, 2 * 8 * B], YDT, tag="out")

                for p in range(4):  # pairs of slots
                    sa, sb_ = 8 * j0 + 2 * p, 8 * j0 + 2 * p + 1
                    ca, cb = int(g[sa]), int(g[sb_])

                    # psum pair: [128, (oc, s2, b)]
                    y_ps = yps_pool.tile([128, 2 * 2 * B], F32, tag="yps")

                    if ca == cb:
                        # fused: 4 matmuls, N=256 (both slots)
                        mm = [(oc, ic) for oc in range(2) for ic in range(2)]
                        for idx, (oc, ic) in enumerate(mm):
                            nc.tensor.matmul(
                                y_ps[:, oc * 256 : oc * 256 + 256],
                                w_block(ca, ic, oc),
                                xmm[:, ic * 1024 + 2 * p * B : ic * 1024 + 2 * p * B + 256],
                                start=(idx == 0),
                                stop=(idx == len(mm) - 1),
                            )
                    else:
                        # boundary pair: per-slot matmuls, N=128
                        mm = [
                            (s_off, w_c, oc, ic)
                            for s_off, w_c in ((0, ca), (1, cb))
                            for oc in range(2)
                            for ic in range(2)
                        ]
                        for idx, (s_off, w_c, oc, ic) in enumerate(mm):
                            nc.tensor.matmul(
                                y_ps[:, oc * 256 + s_off * 128 : oc * 256 + s_off * 128 + 128],
                                w_block(w_c, ic, oc),
                                xmm[:, ic * 1024 + (2 * p + s_off) * B : ic * 1024 + (2 * p + s_off) * B + 128],
                                start=(idx == 0),
                                stop=(idx == len(mm) - 1),
                            )

                    # tanh + fused per-partition bias, psum -> o_sb
                    for oc in range(2):
                        if ca == cb:
                            nc.scalar.activation(
                                o_sb[:, oc * 1024 + 2 * p * B : oc * 1024 + 2 * p * B + 256],
                                y_ps[:, oc * 256 : oc * 256 + 256],
                                TanhF,
                                bias=bias_sb[:, 2 * ca + oc : 2 * ca + oc + 1],
                            )
                        else:
                            for s_off, c in ((0, ca), (1, cb)):
                                nc.scalar.activation(
                                    o_sb[:, oc * 1024 + (2 * p + s_off) * B : oc * 1024 + (2 * p + s_off) * B + 128],
                                    y_ps[:, oc * 256 + s_off * 128 : oc * 256 + s_off * 128 + 128],
                                    TanhF,
                                    bias=bias_sb[:, 2 * c + oc : 2 * c + oc + 1],
                                )

                    if p in (1, 3):
                        h0 = (p // 2) * 4 * B
                        o_h = o_sb[:].rearrange("q (c sb) -> q c sb", c=2)[
                            :, :, h0 : h0 + 4 * B
                        ]
                        x_h = xt_sb[:].rearrange("q (c sb) -> q c sb", c=2)[
                            :, :, h0 : h0 + 4 * B
                        ]
                        nc.vector.tensor_add(o_h, o_h, x_h)


                nc.gpsimd.dma_start(
                    y_d[:, :, 8 * j0 : 8 * j0 + 8, :].rearrange("c p s b -> p c s b"),
                    o_sb[:].rearrange("p (c s b) -> p c s b", c=2, s=8),
                )

    nc.compile()
    return nc


def _get_program(channels: np.ndarray):
    key = (XT_DTYPE, Y_DTYPE, np.asarray(channels).astype(np.int64).tobytes())
    if key not in _CACHE:
        T, deal, g = _plan(channels)
        if T % 8 != 0:
            pad = 8 - T % 8
            deal = np.concatenate([deal, np.repeat(deal[-1:], pad, axis=0)], axis=0)
            g = np.concatenate([g, np.repeat(g[-1:], pad)])
            T += pad
        nc = _build(T, g)
        _CACHE[key] = (T, deal, g, nc)
    return _CACHE[key]


def run(x, channels, weight, bias, trace=False):
    import ml_dtypes

    x = np.ascontiguousarray(np.asarray(x, dtype=np.float32))
    weight = np.asarray(weight, dtype=np.float32)
    bias = np.asarray(bias, dtype=np.float32)

    T, deal, g, nc = _get_program(channels)

    xdt = ml_dtypes.bfloat16 if XT_DTYPE == "bf16" else np.float32
    wt = np.ascontiguousarray(weight.transpose(0, 2, 1)).astype(ml_dtypes.bfloat16)
    # bias -> [o%128, 2*c + oc]
    bias_t = np.ascontiguousarray(
        bias.reshape(N_CH, 2, 128).transpose(2, 0, 1).reshape(128, 2 * N_CH)
    )

    in_maps = []
    for k in range(N_CORES):
        pts = deal[:, k]
        xg = x[:, pts, :]  # [B, T, 256]
        xtk = np.ascontiguousarray(xg.transpose(2, 1, 0)).reshape(2, 128, T, B)
        in_maps.append(
            {
                "xt": xtk.astype(xdt),
                "wt": wt,
                "bias": bias_t,
            }
        )

    res = run_bass_kernel_spmd(nc, in_maps, list(range(N_CORES)), trace=trace)

    y = np.empty((B, N_POINTS, D_OUT), dtype=np.float32)
    for k in range(N_CORES):
        # y^T [2, 128, T, B] -> [B, T, 256]
        ytk = np.asarray(res.results[k]["y"], dtype=np.float32).reshape(D_OUT, T, B)
        y[:, deal[:, k], :] = ytk.transpose(2, 1, 0)
    return y, res


def kernel(x, channels, weight, bias):
    y, _ = run(x, channels, weight, bias, trace=False)
    ch = np.asarray(channels)
    return (y, ch)


# revision 19
# speedup vs baseline: 1.1067x; 1.0604x over previous
"""ChannelDense Trainium2 kernel (nn_ChannelDense_69045894250869).

y[b, n, :] = tanh(x[b, n, :] @ weight[channels[n]].T + bias[channels[n]]) + x[b, n, :]

Shapes: x [128, 2048, 256] f32, channels [2048] int64 (values < 64),
        weight [64, 256, 256] f32, bias [64, 256] f32.
Returns (y [128, 2048, 256] f32, channels passthrough), matching the reference.

Strategy (8 NeuronCores, SPMD single program):
  - Points (n axis) are sorted by channel, each channel's point list is padded
    to a multiple of 8 (repeating real points), and the padded list is dealt
    round-robin across the 8 cores.  Slot t on every core then has the same
    channel, so one static program works for all cores; all per-core
    variability lives in the host-sliced inputs (sharding by point list, with
    channels co-sharded, per the problem's sharding hint).
  - The whole device computation runs in the TRANSPOSED domain: the host
    ships each core's point shard as x^T [ichunk, i%128, slot, b] so the
    contraction axis i lies on SBUF partitions.  Per pair of same-channel
    slots: four bf16 matmuls (stationary = weight block [i,o], moving = x^T
    [i, (slot,b)] with N=256) accumulate z^T in fp32 PSUM; ACT applies
    tanh with the per-partition channel bias fused (z^T has o on partitions);
    DVE adds the x^T residual for a whole 8-slot chunk in one op; results
    store as y^T, which the host un-transposes while unsharding.
  - This keeps the PE free of transpose-mode instructions (which hold the
    HAM clock gate at 1.2 GHz) and needs no on-chip transposes, PSUM->SBUF
    copies, or bias matmuls at all.
  - XT_DTYPE selects what the x^T shard ships as: "bf16" (fastest: half the
    x traffic; the residual also uses bf16-rounded x, rel err ~3e-3) or
    "f32" (exact residual, rel err ~1e-3, more DMA).
"""

import os
import sys

sys.path.insert(0, "/opt/trn_rl_repo")
os.environ.setdefault("NEURON_RT_RESET_CORES", "1")

import numpy as np

import concourse.bass as bass  # noqa: F401
import concourse.mybir as mybir
import concourse.tile as tile
from concourse import bacc
from concourse.bass_utils import run_bass_kernel_spmd

F32 = mybir.dt.float32
BF16 = mybir.dt.bfloat16

XT_DTYPE = os.environ.get("CHD_XT", "bf16")  # "bf16" | "f32"
Y_DTYPE = os.environ.get("CHD_Y", "bf16")  # "bf16" | "f32"

N_CORES = 8
B = 128
N_POINTS = 2048
D_IN = 256
D_OUT = 256
N_CH = 64

_CACHE = {}


def _plan(channels: np.ndarray):
    """Sort points by channel, pad each channel to a multiple of 8, deal
    round-robin over cores.  Returns (slots_per_core T, deal [T,8], g [T])."""
    ch = np.asarray(channels).astype(np.int64)
    per_ch = {}
    for c in range(N_CH):
        pts = np.nonzero(ch == c)[0]
        if len(pts) == 0:
            continue
        pad = (-len(pts)) % N_CORES
        per_ch[c] = np.concatenate([pts, np.repeat(pts[-1], pad)])
    # order channels so even-slot-length runs come first: fewer slot pairs
    # that straddle a channel boundary at odd parity
    order = sorted(per_ch, key=lambda c: (len(per_ch[c]) // N_CORES) % 2)
    padded = []
    slot_ch = []
    for c in order:
        full = per_ch[c]
        padded.append(full)
        slot_ch.extend([c] * (len(full) // N_CORES))
    padded = np.concatenate(padded)
    assert len(padded) % N_CORES == 0
    T = len(padded) // N_CORES
    deal = padded.reshape(T, N_CORES)
    g = np.asarray(slot_ch, dtype=np.int64)
    assert len(g) == T
    return T, deal, g


def _build(T: int, g: np.ndarray):
    assert T % 8 == 0
    xt_bf16 = XT_DTYPE == "bf16"
    XDT = BF16 if xt_bf16 else F32
    YDT = BF16 if Y_DTYPE == "bf16" else F32
    nc = bacc.Bacc("TRN2", target_bir_lowering=False, debug=False, num_devices=N_CORES)

    # x^T shard: [ichunk, i%128, slot, b]
    xt_d = nc.dram_tensor("xt", [2, 128, T, B], XDT, kind="ExternalInput").ap()
    wt_d = nc.dram_tensor("wt", [N_CH, D_IN, D_OUT], BF16, kind="ExternalInput").ap()
    # bias: [o%128, 2*c + ochunk]
    bias_d = nc.dram_tensor("bias", [128, 2 * N_CH], F32, kind="ExternalInput").ap()
    # y^T: [ochunk, o%128, slot, b]
    y_d = nc.dram_tensor("y", [2, 128, T, B], YDT, kind="ExternalOutput").ap()

    TanhF = mybir.ActivationFunctionType.Tanh

    with tile.TileContext(nc) as tc:
        with (
            tc.tile_pool(name="const", bufs=1) as cpool,
            tc.tile_pool(name="xin", bufs=12) as xpool,
            tc.tile_pool(name="xbq", bufs=4) as xbpool,
            tc.tile_pool(name="out", bufs=10) as opool,
            tc.tile_pool(name="yps", bufs=4, space="PSUM") as yps_pool,
        ):
            bias_sb = cpool.tile([128, 2 * N_CH], F32, tag="bias")
            nc.sync.dma_start(bias_sb[:], bias_d)

            # all 64 transposed weights resident: [i%128, (c, ichunk, o)]
            wall_sb = cpool.tile([128, N_CH * 2 * D_OUT], BF16, tag="wall")
            nc.sync.dma_start(
                wall_sb[:].rearrange("p (c i o) -> p c i o", c=N_CH, i=2),
                wt_d.rearrange("c (i p) o -> p c i o", p=128),
            )

            def w_block(c, ic, oc):
                base = c * 512 + ic * 256 + oc * 128
                return wall_sb[:, base : base + 128]

            for j0 in range(T // 8):
                # x^T chunk: [128, (ic, s8, b)]
                xt_sb = xpool.tile([128, 2 * 8 * B], XDT, tag="x")
                nc.sync.dma_start(
                    xt_sb[:].rearrange("p (c s b) -> p c s b", c=2, s=8),
                    xt_d[:, :, 8 * j0 : 8 * j0 + 8, :].rearrange("c p s b -> p c s b"),
                )
                if xt_bf16:
                    xmm = xt_sb
                else:
                    xmm = xbpool.tile([128, 2 * 8 * B], BF16, tag="xb")
                    nc.vector.tensor_copy(xmm[:], xt_sb[:])

                # o_sb: [128, (oc, s8, b)]
                o_sb = opool.tile([128, 2 * 8 * B], YDT, tag="out")

                for q in range(2):  # quads of 4 slots
                    s0 = 8 * j0 + 4 * q
                    cs = [int(g[s0 + i]) for i in range(4)]
                    qb = 4 * q * B  # col base of this quad in (s8, b) space

                    # psum quad: [128, (oc2, s4, b)], oc stride 512 (one bank per oc)
                    y_ps = yps_pool.tile([128, 2 * 4 * B], F32, tag="yps")

                    if cs[0] == cs[1] == cs[2] == cs[3]:
                        # pure quad: per oc, 2 matmuls N=512 spanning all 4 slots
                        for oc in range(2):
                            for ic in range(2):
                                nc.tensor.matmul(
                                    y_ps[:, oc * 512 : oc * 512 + 512],
                                    w_block(cs[0], ic, oc),
                                    xmm[:, ic * 1024 + qb : ic * 1024 + qb + 512],
                                    start=(ic == 0),
                                    stop=(ic == 1),
                                )
                        for oc in range(2):
                            nc.scalar.activation(
                                o_sb[:, oc * 1024 + qb : oc * 1024 + qb + 512],
                                y_ps[:, oc * 512 : oc * 512 + 512],
                                TanhF,
                                bias=bias_sb[:, 2 * cs[0] + oc : 2 * cs[0] + oc + 1],
                            )
                    else:
                        # decompose into two pairs; per oc-bank its own start/stop group
                        for oc in range(2):
                            units = []  # (colbase within oc bank, xcol base, channel, ncols)
                            for pp in range(2):
                                ca, cb = cs[2 * pp], cs[2 * pp + 1]
                                if ca == cb:
                                    units.append((pp * 256, qb + pp * 2 * B, ca, 256))
                                else:
                                    units.append((pp * 256, qb + pp * 2 * B, ca, 128))
                                    units.append((pp * 256 + 128, qb + (pp * 2 + 1) * B, cb, 128))
                            n_mm = len(units) * 2
                            idx = 0
                            for colb, xcb, c, ncols in units:
                                for ic in range(2):
                                    nc.tensor.matmul(
                                        y_ps[:, oc * 512 + colb : oc * 512 + colb + ncols],
                                        w_block(c, ic, oc),
                                        xmm[:, ic * 1024 + xcb : ic * 1024 + xcb + ncols],
                                        start=(idx == 0),
                                        stop=(idx == n_mm - 1),
                                    )
                                    idx += 1
                            for colb, xcb, c, ncols in units:
                                nc.scalar.activation(
                                    o_sb[:, oc * 1024 + xcb : oc * 1024 + xcb + ncols],
                                    y_ps[:, oc * 512 + colb : oc * 512 + colb + ncols],
                                    TanhF,
                                    bias=bias_sb[:, 2 * c + oc : 2 * c + oc + 1],
                                )

                    # residual for this quad (both oc halves), one DVE op
                    o_h = o_sb[:].rearrange("z (c sb) -> z c sb", c=2)[:, :, qb : qb + 512]
                    x_h = xt_sb[:].rearrange("z (c sb) -> z c sb", c=2)[:, :, qb : qb + 512]
                    nc.vector.tensor_add(o_h, o_h, x_h)

                nc.gpsimd.dma_start(
                    y_d[:, :, 8 * j0 : 8 * j0 + 8, :].rearrange("c p s b -> p c s b"),
                    o_sb[:].rearrange("p (c s b) -> p c s b", c=2, s=8),
                )

    nc.compile()
    return nc


def _get_program(channels: np.ndarray):
    key = (XT_DTYPE, Y_DTYPE, np.asarray(channels).astype(np.int64).tobytes())
    if key not in _CACHE:
        T, deal, g = _plan(channels)
        if T % 8 != 0:
            pad = 8 - T % 8
            deal = np.concatenate([deal, np.repeat(deal[-1:], pad, axis=0)], axis=0)
            g = np.concatenate([g, np.repeat(g[-1:], pad)])
            T += pad
        nc = _build(T, g)
        _CACHE[key] = (T, deal, g, nc)
    return _CACHE[key]


def run(x, channels, weight, bias, trace=False):
    import ml_dtypes

    x = np.ascontiguousarray(np.asarray(x, dtype=np.float32))
    weight = np.asarray(weight, dtype=np.float32)
    bias = np.asarray(bias, dtype=np.float32)

    T, deal, g, nc = _get_program(channels)

    xdt = ml_dtypes.bfloat16 if XT_DTYPE == "bf16" else np.float32
    wt = np.ascontiguousarray(weight.transpose(0, 2, 1)).astype(ml_dtypes.bfloat16)
    # bias -> [o%128, 2*c + oc]
    bias_t = np.ascontiguousarray(
        bias.reshape(N_CH, 2, 128).transpose(2, 0, 1).reshape(128, 2 * N_CH)
    )

    in_maps = []
    for k in range(N_CORES):
        pts = deal[:, k]
        xg = x[:, pts, :]  # [B, T, 256]
        xtk = np.ascontiguousarray(xg.transpose(2, 1, 0)).reshape(2, 128, T, B)
        in_maps.append(
            {
                "xt": xtk.astype(xdt),
                "wt": wt,
                "bias": bias_t,
            }
        )

    res = run_bass_kernel_spmd(nc, in_maps, list(range(N_CORES)), trace=trace)

    y = np.empty((B, N_POINTS, D_OUT), dtype=np.float32)
    for k in range(N_CORES):
        # y^T [2, 128, T, B] -> [B, T, 256]
        ytk = np.asarray(res.results[k]["y"], dtype=np.float32).reshape(D_OUT, T, B)
        y[:, deal[:, k], :] = ytk.transpose(2, 1, 0)
    return y, res


def kernel(x, channels, weight, bias):
    y, _ = run(x, channels, weight, bias, trace=False)
    ch = np.asarray(channels)
    return (y, ch)
